# revision 31
# baseline (speedup 1.0000x reference)
"""Trainium2 Bass kernel for nn_ESBN_77352361001553 (scatter_memory).

Math (see the reference's faithfulness note): the conv encoder is dead code
and the LSTM input is constant zeros, so every batch row follows the same
16-step, 512-dim LSTM trajectory from zero state; the (16, 1024, 4) output
is out_t = Wo @ h_t + bo broadcast across batch. Each of the 8 cores runs
the identical recurrence on-chip and emits its own 128-wide batch shard.

Raw Bass (no TileContext), hand-placed semaphores; ~56 us vs the 66.4 us
tile version. What bought the time:
 - One then_inc per 16-matmul gate group instead of one per matmul. The
   tile framework's per-MM increments serialize at ~34 ns on the EVT_SEM
   port while the LDW+MM pairs stream at 27 ns (fp8 FWL floor), building
   a ~450 ns semaphore backlog that delayed every gate activation.
 - One PE wait per step, carried by the first W-matmul of group i
   (h_{t-1}); the four bias matmuls are hoisted ahead of the wait so they
   overlap the previous step's activation tail. All other hazards are
   transitively ordered by engine FIFOs (verified by hand).
 - Gate banks: group gi at step t lives in PSUM bank 4*(t%2)+gi, so no
   engine ever reads a bank the PE is writing (collision-fatal). tanh(cx)
   and sigmoid(f) write PSUM (faster ScE port); warmup MMs never use
   start=True after step 0's biases own their banks.
 - Weights fp8e4 (x64, descale fused into the activation scale), biases
   injected per group by an N=4 matmul (lhsT = bias rows, rhs = I4).
 - DMA: wT as two contiguous group-major 512 KB chunks (i+g, f+o) on the
   SP HWDGE queue with per-chunk gating of step 1; bias aux (32 rows) and
   head aux (128 rows) separately on the Act queue (the whole stream runs
   at the ~190 GB/s fabric ceiling and bounds startup).
 - Output head split: rows 0..14 (need only h_14) are matmul'd, bo-added
   via a K=1 ones matmul, broadcast, and DMA'd during step 15's tail;
   row 15 follows after h_15, so the final barrier + the fixed ~7 us
   end-of-NEFF semaphore sweep start as early as possible.
"""

import os

import numpy as np

T = 16
HID = 512
N_CORES = 8
BSH = 128  # batch shard per core

_BUILT = {}
last_results = None  # BassKernelResults of the most recent run (for tooling)


def _ensure_ntff_hook():
    """Register the axon NTFF profiling hook if the container lacks
    antenv.axon_hooks (slim boot)."""
    import contextlib
    import ctypes
    import sys
    import types

    try:
        from antenv.axon_hooks import get_axon_ntff_profile_hook  # noqa: F401

        return
    except ImportError:
        pass

    so_path = "/opt/axon/libaxon_pjrt.so"
    hook = None
    if os.path.exists(so_path):
        lib = ctypes.CDLL(so_path)
        if hasattr(lib, "axon_start_nrt_profile"):
            lib.axon_start_nrt_profile.argtypes = [
                ctypes.POINTER(ctypes.c_int64),
                ctypes.c_size_t,
            ]
            lib.axon_start_nrt_profile.restype = ctypes.c_int64
            lib.axon_stop_nrt_profile.argtypes = [ctypes.c_char_p]
            lib.axon_stop_nrt_profile.restype = ctypes.c_int64

            @contextlib.contextmanager
            def _hook(output_dir, device_ids):
                import jax

                jax.devices()
                if device_ids:
                    ids = (ctypes.c_int64 * len(device_ids))(*device_ids)
                    rc = lib.axon_start_nrt_profile(ids, len(device_ids))
                else:
                    rc = lib.axon_start_nrt_profile(None, 0)
                if rc != 0:
                    raise RuntimeError(f"axon_start_nrt_profile rc={rc}")
                try:
                    yield
                finally:
                    n = lib.axon_stop_nrt_profile(str(output_dir).encode())
                    print(f"ntff profile: {n} file(s) -> {output_dir}", file=sys.stderr)

            hook = _hook

    mod = types.ModuleType("antenv.axon_hooks")
    mod.get_axon_ntff_profile_hook = lambda: hook
    mod.set_axon_ntff_profile_hook = lambda h: None
    import antenv

    antenv.axon_hooks = mod
    sys.modules["antenv.axon_hooks"] = mod


def _build():
    import concourse.bacc as bacc
    import concourse.bass as bass
    import concourse.mybir as mybir

    f32 = mybir.dt.float32
    f16 = mybir.dt.float16
    f8 = mybir.dt.float8e4
    AF = mybir.ActivationFunctionType

    nc = bacc.Bacc("TRN2", target_bir_lowering=False, debug=False, enable_asserts=False)

    wT_d = nc.dram_tensor("wT", [2, 128, 4096], f8, kind="ExternalInput")
    auxa_d = nc.dram_tensor("auxa", [32, 516], f16, kind="ExternalInput")
    auxb_d = nc.dram_tensor("auxb", [128, 36], f16, kind="ExternalInput")
    out_d = nc.dram_tensor("out", [T, BSH, 4], f32, kind="ExternalOutput")

    # SBUF (persistent allocations; no pools needed for a fixed kernel)
    wT = nc.alloc_sbuf_tensor("wTs", [128, 8192], f8)
    aux = nc.alloc_sbuf_tensor("auxs", [128, 552], f16)
    hs = nc.alloc_sbuf_tensor("hss", [128, 4 * T], f16)
    cx = nc.alloc_sbuf_tensor("cxs", [128, 4], f32)
    si = nc.alloc_sbuf_tensor("sis", [128, 4], f16)
    tg = nc.alloc_sbuf_tensor("tgs", [128, 4], f16)
    sf = nc.alloc_sbuf_tensor("sfs", [128, 4], f16)
    so = nc.alloc_sbuf_tensor("sos", [128, 4], f16)
    th = nc.alloc_sbuf_tensor("ths", [128, 4], f16)
    t1 = nc.alloc_sbuf_tensor("t1s", [128, 4], f32)
    cxa = nc.alloc_sbuf_tensor("cxas", [128, 4], f32)
    wz = nc.alloc_sbuf_tensor("wzs", [128, 128], f16)  # warmup lhsT (garbage ok)
    # scan-fused cell state: two parity buffers + zero-interleaved sigma_f
    sA = nc.alloc_sbuf_tensor("scanA", [128, 10], f32)
    sB = nc.alloc_sbuf_tensor("scanB", [128, 10], f32)
    sfz = nc.alloc_sbuf_tensor("sfz7", [128, 7], f16)
    w2 = nc.alloc_sbuf_tensor("w2s", [128, 4], f16)  # ACT-table warm scratch
    head = nc.alloc_sbuf_tensor("heads", [16, 4], f32)
    headb = nc.alloc_sbuf_tensor("headbs", [1, 4], f32)
    bc = nc.alloc_sbuf_tensor("bcs", [16, 512], f32)
    bcb = nc.alloc_sbuf_tensor("bcbs", [1, 512], f32)

    id4 = aux[:, 512:516]

    # PSUM: all 8 banks; group gi at step t -> bank 4*(t%2)+gi, cols 0:4.
    pb = [nc.alloc_psum_tensor(f"pb{i}", [128, 512], f32) for i in range(8)]

    def bank(t, gi):
        return pb[4 * (t % 2) + gi]

    s_dma_a = nc.alloc_semaphore("s_dma_a")  # wT groups i+g (SP queue)
    s_dma_c = nc.alloc_semaphore("s_dma_c")  # wT groups f+o (SP queue)
    s_dma_b = nc.alloc_semaphore("s_dma_b")  # aux_a (Act queue)
    s_dma_d = nc.alloc_semaphore("s_dma_d")  # aux_b (Act queue)
    s_pe = nc.alloc_semaphore("s_pe")  # PE gate-group completions
    s_act = nc.alloc_semaphore("s_act")  # scalar ACT completions
    s_dve = nc.alloc_semaphore("s_dve")  # vector completions
    s_out = nc.alloc_semaphore("s_out")  # output DMA

    # ---- static tick tables ------------------------------------------------
    # s_pe: t=0 emits bias i,g,o (f skipped: cx_0 = i*g); t>=1: 4 groups.
    pe_tick = {}
    n = 0
    for gi in (0, 1, 3):
        n += 1
        pe_tick[(0, gi)] = n
    for t in range(1, T):
        for gi in range(4):
            n += 1
            pe_tick[(t, gi)] = n
    n += 1
    pe_tick["head"] = n

    # s_act: per step sigma_i, tanh_g, [sigma_f], sigma_o, tanh_cx
    act_tick = {}
    n = 0
    for t in range(T):
        for key in ("i", "g", "f", "o", "h"):
            if t == 0 and key == "f":
                continue
            n += 1
            act_tick[(t, key)] = n

    # s_dve tick order mirrors the DVE FIFO exactly:
    #   auxz, wz, [steps 0..14], then step 15 with the split-head A ops
    #   (copyA/bcA) slotted between cx(15) and h(15), then copyB/bcB.
    dve_tick = {}
    dve_tick["auxz"] = 1
    dve_tick["wz"] = 2
    n = 2
    for t in range(T):
        for key in (("cx", "h") if t == 0 else ("cxa", "cx", "h")):
            if t == T - 1 and key == "h":
                n += 1
                dve_tick["cpA"] = n
            n += 1
            dve_tick[(t, key)] = n
    for key in ("bcA", "bcB"):
        n += 1
        dve_tick[key] = n

    # s_pe extra ticks for the split head
    pe_tick["headA"] = pe_tick["head"]
    pe_tick["headB"] = pe_tick["head"] + 1

    def wtile(gi, c, ko):
        i0 = ((4 * gi + c) * 4 + ko) * 128
        return wT[:, i0 : i0 + 128]

    # ---- main block --------------------------------------------------------
    # (no explicit sem clears: the Bass preamble range-clears the kernel
    # semaphore range on every run)
    with nc.Block("main", no_gpsimd_drain=True) as blk:

        @blk.sync
        def _(sync):
            sync.dma_start(wT[:, 0:4096], wT_d[0]).then_inc(s_dma_a, 16)
            sync.dma_start(wT[:, 4096:8192], wT_d[1]).then_inc(s_dma_c, 16)
            out2 = out_d.rearrange("t b d -> t (b d)")
            # no completion waits: the end-of-block drain flushes the queue,
            # and the DMA receipts overlap the fixed epilogue
            sync.wait_ge(s_dve, dve_tick["bcA"])
            sync.dma_start(out2[0:15, :], bc[0:15, :]).then_inc(s_out, 16)
            sync.wait_ge(s_dve, dve_tick["bcB"])
            sync.dma_start(out2[15:16, :], bcb[:]).then_inc(s_out, 16)

        @blk.scalar
        def _(scalar):
            scalar.dma_start(aux[0:32, 0:516], auxa_d[:]).then_inc(s_dma_b, 16)
            scalar.dma_start(aux[:, 516:552], auxb_d[:]).then_inc(s_dma_d, 16)
            # preload both ACT tables (input zeros; output scratch)
            scalar.wait_ge(s_dve, dve_tick["wz"])
            scalar.activation(w2[:, 0:2], wz[:, 0:2], AF.Sigmoid)
            scalar.activation(w2[:, 2:4], wz[:, 2:4], AF.Tanh)
            for t in range(T):
                scalar.activation(
                    si[:], bank(t, 0)[:, 0:4], AF.Sigmoid, scale=1.0 / 64.0
                )._wait_ge(s_pe, pe_tick[(t, 0)]).then_inc(s_act)
                scalar.activation(
                    tg[:], bank(t, 1)[:, 0:4], AF.Tanh, scale=1.0 / 64.0
                )._wait_ge(s_pe, pe_tick[(t, 1)]).then_inc(s_act)
                if t > 0:
                    scalar.activation(
                        sfz[:, 0:7:2], bank(t, 2)[:, 0:4],
                        AF.Sigmoid, scale=1.0 / 64.0,
                    )._wait_ge(s_pe, pe_tick[(t, 2)]).then_inc(s_act)
                scalar.activation(
                    so[:], bank(t, 3)[:, 0:4], AF.Sigmoid, scale=1.0 / 64.0
                )._wait_ge(s_pe, pe_tick[(t, 3)]).then_inc(s_act)
                thsrc = sA[:, 0:7:2] if t % 2 == 1 else sB[:, 0:7:2]
                scalar.activation(
                    bank(t, 0)[:, 8:12], thsrc, AF.Tanh
                )._wait_ge(s_dve, dve_tick[(t, "cx")]).then_inc(s_act)

        @blk.tensor
        def _(tensor):
            def warm(k, first_start=False):
                # start=False: never clears a bank (bank 3 col 16 is scratch);
                # only the very first warmup MM opens the bank.
                for j in range(k):
                    tensor.matmul(
                        pb[3][:, 16:17],
                        wz[:],
                        wz[:, 0:1],
                        start=(first_start and j == 0),
                        stop=False,
                        skip_group_check=True,
                    )

            def bias_mm(t, gi, stop):
                return tensor.matmul(
                    bank(t, gi)[:, 0:4],
                    aux[:, gi * 128 : gi * 128 + 128],
                    id4,
                    start=True,
                    stop=stop,
                    skip_group_check=True,
                )

            tensor.wait_ge(s_dve, dve_tick["wz"])  # wz+aux zeros done
            # all warmups strictly precede the step-0 bias MMs: a warm MM
            # writing bank 3 while sigma_o(0) reads it would be a fatal
            # PSUM bank collision
            warm(45, first_start=True)
            # step 0: gates are pure bias (h_{-1} = 0); group f unused
            bias_mm(0, 0, True)._wait_ge(s_dma_b, 16).then_inc(s_pe)  # aux_a
            bias_mm(0, 1, True).then_inc(s_pe)
            bias_mm(0, 3, True).then_inc(s_pe)
            for t in range(1, T):
                # all 4 bias MMs first: they depend on nothing recent, so
                # they overlap the previous step's activation tail
                for gi in range(4):
                    bias_mm(t, gi, False)
                for gi in range(4):
                    if t == 1 and gi == 0:
                        tensor.wait_ge(s_dma_a, 16)  # wT groups i+g
                    if t == 1 and gi == 2:
                        tensor.wait_ge(s_dma_c, 16)  # wT groups f+o
                    for c in range(4):
                        for ko in range(4):
                            mm = tensor.matmul(
                                bank(t, gi)[:, c : c + 1],
                                wtile(gi, c, ko),
                                hs[:, 4 * (t - 1) + ko : 4 * (t - 1) + ko + 1],
                                start=False,
                                stop=(c == 3 and ko == 3),
                                skip_group_check=True,
                            )
                            if gi == 0 and c == 0 and ko == 0:
                                mm._wait_ge(s_dve, dve_tick[(t - 1, "h")])
                    mm.then_inc(s_pe)
            # ---- split head ----
            # part A (rows t=0..14) needs only h_14, which step 15's W-MMs
            # already waited on; runs during step 15's activation tail
            tensor.wait_ge(s_dma_d, 16)  # aux_b (head weights)
            for ko in range(4):
                tensor.matmul(
                    pb[3][0:15, 20:24],
                    hs[:, ko : ko + 4 * (T - 2) + 1 : 4],
                    aux[:, 516 + 4 * ko : 520 + 4 * ko],
                    start=(ko == 0),
                    stop=False,
                    skip_group_check=True,
                )
            # bo row: out[t, d] += 1 * bo[d]  (K=1 matmul)
            tensor.matmul(
                pb[3][0:15, 20:24],
                aux[0:1, 532:547],
                aux[0:1, 548:552],
                start=False,
                stop=True,
                skip_group_check=True,
            ).then_inc(s_pe)  # headA
            # dummy MM carries the h_15 wait so part B's LDWEIGHTS (which
            # read hs col 60+) cannot be pulled ahead of it
            tensor.matmul(
                pb[3][:, 16:17],
                wz[:],
                wz[:, 0:1],
                start=False,
                stop=False,
                skip_group_check=True,
            )._wait_ge(s_dve, dve_tick[(T - 1, "h")])
            for ko in range(4):
                tensor.matmul(
                    pb[3][0:1, 28:32],
                    hs[:, 4 * (T - 1) + ko : 4 * (T - 1) + ko + 1],
                    aux[:, 516 + 4 * ko : 520 + 4 * ko],
                    start=(ko == 0),
                    stop=False,
                    skip_group_check=True,
                )
            tensor.matmul(
                pb[3][0:1, 28:32],
                aux[0:1, 532:533],
                aux[0:1, 548:552],
                start=False,
                stop=True,
                skip_group_check=True,
            ).then_inc(s_pe)  # headB

        @blk.vector
        def _(vector):
            vector.memset(aux[32:64, 0:516], 0.0)
            vector.memset(aux[64:96, 0:516], 0.0)
            vector.memset(aux[96:128, 0:516], 0.0).then_inc(s_dve)
            vector.memset(sA[:], 0.0)
            vector.memset(sB[:], 0.0)
            vector.memset(sfz[:], 0.0)
            vector.memset(wz[:], 0.0).then_inc(s_dve)
            for t in range(T):
                if t == 0:
                    vector.tensor_mul(sB[:, 0:7:2], si[:], tg[:])._wait_ge(
                        s_act, act_tick[(0, "g")]
                    ).then_inc(s_dve)
                else:
                    # cx_t = sigma_f (.) cx' + sigma_i (.) tanh_g, fused as one
                    # 7-col prefix-scan: [f0,0,f1,0,f2,0,f3] (x) state +
                    # [t1_0, cx'_1, t1_1, cx'_2, t1_2, cx'_3, t1_3], with
                    # initial = cx'_0. Results land at even cols of the
                    # destination buffer = exactly where the next step reads
                    # its cx' from, so the form self-composes (A/B alternate).
                    prev, cur = (sB, sA) if t % 2 == 1 else (sA, sB)
                    vector.tensor_mul(prev[:, 1:8:2], si[:], tg[:])._wait_ge(
                        s_act, act_tick[(t, "g")]
                    ).then_inc(s_dve)
                    vector.wait_ge(s_dve, dve_tick[(t, "cxa")])
                    vector.tensor_tensor_scan(
                        cur[:, 0:7], sfz[:, 0:7], prev[:, 1:8],
                        prev[:, 0:1], mybir.AluOpType.mult,
                        mybir.AluOpType.add,
                    )._wait_ge(s_act, act_tick[(t, "f")]).then_inc(s_dve)
                if t == T - 1:
                    # copyA slots before h(15): its PE data (headA) lands
                    # well before h's, and FIFO order lets headB's later
                    # pb[3] writes never collide with this read
                    vector.tensor_copy(head[0:15, :], pb[3][0:15, 20:24])._wait_ge(
                        s_pe, pe_tick["headA"]
                    ).then_inc(s_dve)
                vector.tensor_mul(
                    hs[:, 4 * t : 4 * t + 4], so[:], bank(t, 0)[:, 8:12]
                )._wait_ge(s_act, act_tick[(t, "h")]).then_inc(s_dve)
            hap = head[0:15, :]
            rep = bass.AP(hap.tensor, hap.offset, [list(hap.ap[0]), [0, BSH], [1, 4]])
            vector.tensor_copy(
                bc[0:15, :].rearrange("t (b d) -> t b d", d=4), rep
            )._wait_ge(s_dve, dve_tick["cpA"]).then_inc(s_dve)  # bcA
            hbp = pb[3][0:1, 28:32]
            repb = bass.AP(hbp.tensor, hbp.offset, [list(hbp.ap[0]), [0, BSH], [1, 4]])
            vector.tensor_copy(
                bcb[:].rearrange("t (b d) -> t b d", d=4), repb
            )._wait_ge(s_pe, pe_tick["headB"]).then_inc(s_dve)  # bcB from psum

    nc.compile()
    return nc


def prep_inputs(Whh, bih, bhh, Wo, bo):
    """Host-side weight relayout (all tensors are tiny: <5 MB total)."""
    Whh = np.asarray(Whh, np.float64)
    c = np.asarray(bih, np.float64) + np.asarray(bhh, np.float64)
    Wo = np.asarray(Wo, np.float32)
    bo = np.asarray(bo, np.float32)
    H = HID
    # gate order i, g, f, o (torch rows: i, f, g, o)
    perm = np.concatenate(
        [
            np.arange(0, H),
            np.arange(2 * H, 3 * H),
            np.arange(H, 2 * H),
            np.arange(3 * H, 4 * H),
        ]
    )
    Wp = (Whh[perm] * 64.0).astype(np.float32)
    cp = (c[perm] * 64.0).astype(np.float32)
    # tile-major interleave: tile (jo, ko) at cols (jo*4+ko)*128,
    # value wT[p, .*128+m] = W_perm[jo*128+m, ko*128+p]
    import ml_dtypes

    wT = np.ascontiguousarray(
        Wp.reshape(16, 128, 4, 128).transpose(3, 0, 2, 1).reshape(128, 8192)
    ).astype(ml_dtypes.float8_e4m3)
    # bias tiles: row k holds the biases of psum column k of the group
    wbias = np.zeros((128, 512), np.float32)
    cpr = cp.reshape(4, 4, 128)  # [gi, k, m]
    for gi in range(4):
        wbias[0:4, gi * 128 : (gi + 1) * 128] = cpr[gi]
    id4 = np.zeros((128, 4), np.float32)
    id4[np.arange(4), np.arange(4)] = 1.0
    woT = np.ascontiguousarray(
        Wo.reshape(4, 4, 128).transpose(2, 1, 0).reshape(128, 16)
    )
    ones16 = np.zeros((128, 16), np.float32)
    ones16[0, :] = 1.0
    bo4 = np.zeros((128, 4), np.float32)
    bo4[0, :] = bo
    auxa = np.concatenate([wbias, id4], axis=1).astype(np.float16)[0:32]  # [32, 516]
    auxb = np.concatenate([woT, ones16, bo4], axis=1).astype(np.float16)  # [128, 36]
    # group-major halves: chunk 0 = gate groups i+g, chunk 1 = f+o, each a
    # contiguous 512 KB DMA
    wT2 = np.ascontiguousarray(wT.reshape(128, 2, 4096).transpose(1, 0, 2))
    return {"wT": wT2, "auxa": np.ascontiguousarray(auxa), "auxb": np.ascontiguousarray(auxb)}


def kernel(**inputs) -> np.ndarray:
    global last_results
    from concourse.bass_utils import run_bass_kernel_spmd

    if "nc" not in _BUILT:
        _BUILT["nc"] = _build()
    nc = _BUILT["nc"]

    in_map = prep_inputs(
        inputs["Whh"], inputs["bih"], inputs["bhh"], inputs["Wo"], inputs["bo"]
    )
    if os.environ.get("BASS_TRACE"):
        _ensure_ntff_hook()
    in_maps = [dict(in_map) for _ in range(N_CORES)]
    res = run_bass_kernel_spmd(
        nc,
        in_maps,
        core_ids=list(range(N_CORES)),
        trace=bool(os.environ.get("BASS_TRACE")),
    )
    last_results = res
    return np.concatenate([r["out"] for r in res.results], axis=1)
